# Initial kernel scaffold
#
"""Trainium2 Bass kernel for the GAT problem (nn_GAT_42786464203341).

Strategy (8-way tensor parallel, Megatron-style):
  - The "GAT" edges are block-diagonal fully-connected per sample, so the
    message passing is dense per-sample attention (scores are rank-2
    broadcast sums el[i] + er[j], leaky-relu'd, softmaxed over src i).
  - Activations are kept feature-major (x^T: [D, nodes]) on every core so
    they can feed the PE array directly as lhsT/rhs without transposes.
    LayerNorm reductions (over D = partitions) are done with ones-vector
    matmuls; per-node stats are broadcast back across partitions with
    rank-1 (K=1) matmuls.
  - Attention is head-parallel: each core owns 2 of the 16 heads
    (W_attn / a_l / a_r column shards).  W_proj is row-sharded so each
    core produces a partial [D, nodes] contribution -> AllReduce.
    The FFN is column/row sharded (W_fc cols, W_out rows) -> AllReduce.
    The LM head is vocab-sharded; each core returns a [1024, 530] logits
    slice and the host concatenates (no device gather needed).
  - Matmuls run in float32r (tf32-like, ~1.5e-4 rel err, full PE rate at
    moving-free >= 256).  Node axis is padded 530 -> 532 (two 266-wide
    batch chunks) to satisfy the f32r even-free-dim constraint.
  - 12 AllReduces total (3 layers x 2 sublayers x 2 batch chunks),
    batch-split so collectives overlap the other batch's compute.
"""

import numpy as np

import concourse.bass as bass
import concourse.tile as tile
from concourse import bacc, mybir
from concourse.masks import make_identity

F32 = mybir.dt.float32
F32R = mybir.dt.float32r

B, T, NOBJ = 2, 265, 9
D, H, DH = 1536, 16, 96
V, PV, L, FF = 8192, 512, 3, 6144
N = B * T          # 530
NC = 8             # cores
HPC = H // NC      # heads per core = 2
FFL = FF // NC     # 768
VL = V // NC       # 1024
NCH = T + 1        # 266: per-batch node chunk (col 265 is a zero pad)
NP = B * NCH       # 532
KD = D // 128      # 12 k-tiles over D
KF = FFL // 128    # 6 k-tiles over local FF
MT = [(0, 128), (128, 128), (256, 10)]   # (start, size) node tiles per batch
MT_REAL = [128, 128, 9]                  # real (non-pad) rows per node tile
EPS = 1e-5

_CACHE = {}


# --------------------------------------------------------------------------
# host-side helpers
# --------------------------------------------------------------------------

def _block_diag_edges_np():
    base = np.arange(T)
    src = np.concatenate([g * T + np.repeat(base, T) for g in range(B)])
    dst = np.concatenate([g * T + np.tile(base, T) for g in range(B)])
    return src.astype(np.int64), dst.astype(np.int64)


def _host_inputs(inp):
    """Build the per-core input maps from the full problem inputs."""
    f32 = np.float32
    objs_e = np.asarray(inp["obj_emb_w"])[np.asarray(inp["objs"])]
    pe = np.asarray(inp["poss_emb_w"])[np.asarray(inp["poss"])]
    nfeat = np.concatenate([objs_e, pe[:, :NOBJ], pe[:, NOBJ:]], axis=-1)
    z = np.asarray(inp["tok_emb"])[np.asarray(inp["z_indices"])]
    x0 = np.concatenate([nfeat, z], axis=1) + np.asarray(inp["pos_emb"])[:, :T]
    x0 = x0.reshape(N, D).astype(f32)

    x0t = np.zeros((D, NP), f32)
    for b in range(B):
        x0t[:, b * NCH:b * NCH + T] = x0[b * T:(b + 1) * T].T

    W_attn = np.asarray(inp["W_attn"], f32)
    a_l = np.asarray(inp["a_l"], f32)
    a_r = np.asarray(inp["a_r"], f32)
    W_proj = np.asarray(inp["W_proj"], f32)
    W_fc = np.asarray(inp["W_fc"], f32)
    W_out = np.asarray(inp["W_out"], f32)
    head_w = np.asarray(inp["head_w"], f32)

    def cols(vec_3xD, k_tiles):  # [3, D'] -> [3, 128, k_tiles] column tiles
        v = np.asarray(vec_3xD, f32)
        return np.transpose(v.reshape(3, k_tiles, 128), (0, 2, 1)).copy()

    ln1g = cols(inp["ln1_g"], KD)
    ln1b = cols(inp["ln1_b"], KD)
    ln2g = cols(inp["ln2_g"], KD)
    ln2b = cols(inp["ln2_b"], KD)
    lnfg = np.asarray(inp["lnf_g"], f32).reshape(KD, 128).T.copy()
    lnfb = np.asarray(inp["lnf_b"], f32).reshape(KD, 128).T.copy()
    bfc = cols(inp["b_fc"], KF * NC)      # [3, 128, 48]; core slice below
    bout8 = cols(np.asarray(inp["b_out"], f32) / NC, KD)
    bproj8 = cols(np.asarray(inp["b_proj"], f32) / NC, KD)

    maps = []
    for c in range(NC):
        h0 = c * HPC
        wattn = np.zeros((L, D, 256), f32)
        for j in range(HPC):
            hg = h0 + j
            wattn[:, :, j * DH:(j + 1) * DH] = W_attn[:, :, hg * DH:(hg + 1) * DH]
            wattn[:, hg * DH:(hg + 1) * DH, 192 + j] = a_l[:, hg, :]
            wattn[:, hg * DH:(hg + 1) * DH, 194 + j] = a_r[:, hg, :]
        wproj = np.stack(
            [W_proj[:, (h0 + j) * DH:(h0 + j + 1) * DH, :] for j in range(HPC)],
            axis=1,
        )  # [3, 2, 96, 1536]
        m = {
            "x0t": x0t,
            "wattn": wattn,
            "wproj": np.ascontiguousarray(wproj),
            "wfc": np.ascontiguousarray(W_fc[:, :, c * FFL:(c + 1) * FFL]),
            "wout": np.ascontiguousarray(W_out[:, c * FFL:(c + 1) * FFL, :]),
            "whead": np.ascontiguousarray(head_w[:, c * VL:(c + 1) * VL]),
            "ones_col": np.ones((128, 1), f32),
            "ones_row": np.ones((1, 128), f32),
            "ln1g": ln1g, "ln1b": ln1b, "ln2g": ln2g, "ln2b": ln2b,
            "lnfg": lnfg, "lnfb": lnfb,
            "bfc": np.ascontiguousarray(bfc[:, :, c * KF:(c + 1) * KF]),
            "bout8": bout8, "bproj8": bproj8,
        }
        maps.append(m)
    return maps


# --------------------------------------------------------------------------
# device program
# --------------------------------------------------------------------------

def _build_nc():
    nc = bacc.Bacc("TRN2", target_bir_lowering=False, debug=False, num_devices=NC)

    d_x0t = nc.declare_dram_parameter("x0t", [D, NP], F32R, isOutput=False)
    d_wattn = nc.declare_dram_parameter("wattn", [L, D, 256], F32R, isOutput=False)
    d_wproj = nc.declare_dram_parameter("wproj", [L, HPC, DH, D], F32R, isOutput=False)
    d_wfc = nc.declare_dram_parameter("wfc", [L, D, FFL], F32R, isOutput=False)
    d_wout = nc.declare_dram_parameter("wout", [L, FFL, D], F32R, isOutput=False)
    d_whead = nc.declare_dram_parameter("whead", [D, VL], F32R, isOutput=False)
    d_ones_col = nc.declare_dram_parameter("ones_col", [128, 1], F32R, isOutput=False)
    d_ones_row = nc.declare_dram_parameter("ones_row", [1, 128], F32R, isOutput=False)
    d_ln1g = nc.declare_dram_parameter("ln1g", [L, 128, KD], F32, isOutput=False)
    d_ln1b = nc.declare_dram_parameter("ln1b", [L, 128, KD], F32, isOutput=False)
    d_ln2g = nc.declare_dram_parameter("ln2g", [L, 128, KD], F32, isOutput=False)
    d_ln2b = nc.declare_dram_parameter("ln2b", [L, 128, KD], F32, isOutput=False)
    d_lnfg = nc.declare_dram_parameter("lnfg", [128, KD], F32, isOutput=False)
    d_lnfb = nc.declare_dram_parameter("lnfb", [128, KD], F32, isOutput=False)
    d_bfc = nc.declare_dram_parameter("bfc", [L, 128, KF], F32, isOutput=False)
    d_bout8 = nc.declare_dram_parameter("bout8", [L, 128, KD], F32, isOutput=False)
    d_bproj8 = nc.declare_dram_parameter("bproj8", [L, 128, KD], F32, isOutput=False)
    d_logits = nc.declare_dram_parameter("logits", [VL, N], F32, isOutput=True)

    # AllReduce bounce buffers: [layer][sublayer][batch]
    ar_in, ar_out = {}, {}
    for l in range(L):
        for s in range(2):
            for b in range(B):
                ar_in[l, s, b] = nc.dram_tensor(f"arin_{l}_{s}_{b}", [D, T], F32)
                ar_out[l, s, b] = nc.dram_tensor(
                    f"arout_{l}_{s}_{b}", [D, T], F32, addr_space="Shared"
                )

    AF = mybir.ActivationFunctionType
    ALU = mybir.AluOpType

    with tile.TileContext(nc) as tc:
        pools = {}

        def pool(name, **kw):
            p = tc.tile_pool(name=name, **kw)
            pools[name] = p.__enter__()
            return pools[name]

        res = pool("res", bufs=1)        # persistent (xt, consts)
        act = pool("act", bufs=1)        # h / sq / lnt / e / parts etc (tagged)
        wgt = pool("wgt", bufs=1)        # weight streams (tagged)
        ps = pool("ps", bufs=1, space="PSUM")

        # ---- persistent tiles
        xt = []
        for k in range(KD):
            t = res.tile([128, NP], F32R, tag=f"xt{k}")
            nc.sync.dma_start(out=t[:], in_=d_x0t[k * 128:(k + 1) * 128, :])
            xt.append(t)
        ones_col = res.tile([128, 1], F32R, tag="ones_col")
        nc.sync.dma_start(out=ones_col[:], in_=d_ones_col[:])
        ones_row = res.tile([1, 128], F32R, tag="ones_row")
        nc.sync.dma_start(out=ones_row[:], in_=d_ones_row[:])
        ident = res.tile([128, 128], F32, tag="ident")
        make_identity(nc, ident[:])
        eps_col = res.tile([1, 1], F32, tag="eps")
        nc.vector.memset(eps_col[:], EPS)

        # gains/bias columns per layer (loaded fresh each layer)
        def load_gb(dram, l, ktiles, tag):
            t = act.tile([128, ktiles], F32, tag=tag)
            nc.sync.dma_start(out=t[:], in_=dram[l] if l is not None else dram[:])
            return t

        # ---- LayerNorm (feature-major). returns list of 12 f32r h-tiles
        def layer_norm(lbl, src_tiles, g_sb, b_sb, b):
            cs = b * NCH  # chunk col start
            p_sums = ps.tile([1, NCH], F32, tag="row")
            p_sqs = ps.tile([1, NCH], F32, tag="row")
            sq_tiles = []
            for k in range(KD):
                sq = act.tile([128, NCH], F32R, tag=f"sq{k % 2}")
                nc.scalar.activation(
                    sq[:], src_tiles[k][:, cs:cs + NCH].bitcast(F32), AF.Square
                )
                sq_tiles.append(sq)
                nc.tensor.matmul(
                    p_sums[:], ones_col[:], src_tiles[k][:, cs:cs + NCH],
                    start=(k == 0), stop=(k == KD - 1),
                )
                nc.tensor.matmul(
                    p_sqs[:], ones_col[:], sq[:],
                    start=(k == 0), stop=(k == KD - 1),
                )
            # stats rows
            m_row = act.tile([1, NCH], F32R, tag="m_row")
            nc.vector.tensor_scalar(
                m_row[:], p_sums[:], 1.0 / D, None, ALU.mult
            )
            ms = act.tile([1, NCH], F32, tag="ms_row")
            nc.scalar.activation(ms[:], m_row[:].bitcast(F32), AF.Square)
            var = act.tile([1, NCH], F32, tag="var_row")
            nc.vector.tensor_scalar(var[:], p_sqs[:], 1.0 / D, None, ALU.mult)
            nc.vector.tensor_sub(var[:], var[:], ms[:])
            std = act.tile([1, NCH], F32, tag="std_row")
            nc.scalar.activation(std[:], var[:], AF.Sqrt, bias=eps_col[:])
            rs_row = act.tile([1, NCH], F32R, tag="rs_row")
            with nc.allow_low_precision("f32r rounding is fine"):
                nc.vector.reciprocal(rs_row[:], std[:])
            mr_row = act.tile([1, NCH], F32R, tag="mr_row")
            with nc.allow_low_precision("f32r rounding is fine"):
                nc.vector.tensor_mul(
                    mr_row[:], m_row[:].bitcast(F32), rs_row[:].bitcast(F32)
                )
            # broadcast stats across partitions
            p_rb = ps.tile([128, NCH], F32, tag="bc")
            nc.tensor.matmul(p_rb[:], ones_row[:], rs_row[:], start=True, stop=True)
            p_mb = ps.tile([128, NCH], F32, tag="bc")
            nc.tensor.matmul(p_mb[:], ones_row[:], mr_row[:], start=True, stop=True)
            # apply
            h_tiles = []
            for k in range(KD):
                t1 = act.tile([128, NCH], F32, tag=f"lnt{k % 2}")
                nc.vector.tensor_mul(
                    t1[:], src_tiles[k][:, cs:cs + NCH].bitcast(F32), p_rb[:]
                )
                nc.vector.tensor_sub(t1[:], t1[:], p_mb[:])
                h = act.tile([128, NCH], F32R, tag=f"h{k}")
                nc.scalar.activation(
                    h[:], t1[:], AF.Identity,
                    bias=b_sb[:, lbl * 0 + k:k + 1], scale=g_sb[:, k:k + 1],
                )
                h_tiles.append(h)
            return h_tiles

        # ---- matmul helper: out_tiles[m] = act_consumer(psum) over M tiles
        # lhsT_fn(k, mslice) , rhs = h tiles
        def big_mm(n_mtiles, kt, lhsT_fn, rhs_tiles, consume, nfree=NCH):
            for mi in range(n_mtiles):
                p = ps.tile([128, nfree], F32, tag="mm")
                for k in range(kt):
                    nc.tensor.matmul(
                        p[:, :], lhsT_fn(k, mi), rhs_tiles[k][:],
                        start=(k == 0), stop=(k == kt - 1),
                    )
                consume(mi, p)

        # ================= main network =================
        for l in range(L):
            ln1g_sb = load_gb(d_ln1g, l, KD, "ln1g")
            ln1b_sb = load_gb(d_ln1b, l, KD, "ln1b")
            ln2g_sb = load_gb(d_ln2g, l, KD, "ln2g")
            ln2b_sb = load_gb(d_ln2b, l, KD, "ln2b")
            bfc_sb = load_gb(d_bfc, l, KF, "bfc")
            bout8_sb = load_gb(d_bout8, l, KD, "bout8")
            bproj8_sb = load_gb(d_bproj8, l, KD, "bproj8")

            # attention weights for this layer
            wa = []
            for k in range(KD):
                t = wgt.tile([128, 256], F32R, tag=f"wa{k}")
                nc.sync.dma_start(out=t[:], in_=d_wattn[l, k * 128:(k + 1) * 128, :])
                wa.append(t)
            wp = []
            for j in range(HPC):
                t = wgt.tile([DH, D], F32R, tag=f"wp{j}")
                nc.sync.dma_start(out=t[:], in_=d_wproj[l, j])
                wp.append(t)

            # ---------- sublayer 0: attention ----------
            for b in range(B):
                cs = b * NCH
                h_tiles = layer_norm(l, xt, ln1g_sb, ln1b_sb, b)

                # Wh (node-major) + el/er columns;  whsb[mt]: [128, 198]
                whsb = []
                for mi, (ms, msz) in enumerate(MT):
                    p = ps.tile([128, 256], F32, tag="mm")
                    for k in range(KD):
                        nc.tensor.matmul(
                            p[:msz, :], h_tiles[k][:, ms:ms + msz], wa[k][:],
                            start=(k == 0), stop=(k == KD - 1),
                        )
                    w = act.tile([128, 198], F32R, tag=f"whsb{mi}_{b % 2}")
                    nc.scalar.copy(w[:msz, 0:196], p[:msz, 0:196])
                    nc.vector.tensor_scalar(
                        w[:msz, 196:198], p[:msz, 192:194], 0.2, None, ALU.mult
                    )
                    whsb.append(w)

                # er rows via PE transpose of the er columns
                erow = act.tile([2, NCH], F32R, tag=f"erow{b % 2}")
                for mi, (ms, msz) in enumerate(MT):
                    pt = ps.tile([2, 128], F32, tag="row")
                    nc.tensor.transpose(
                        pt[:, :msz], whsb[mi][:msz, 194:196].bitcast(F32),
                        ident[:msz, :msz],
                    )
                    nc.scalar.copy(erow[:, ms:ms + msz], pt[:, :msz].bitcast(F32))

                # per-head attention
                aggt = []
                for j in range(HPC):
                    p_er = ps.tile([128, NCH], F32, tag="bc")
                    nc.tensor.matmul(
                        p_er[:], ones_row[:], erow[j:j + 1, :], start=True, stop=True
                    )
                    e_tiles = []
                    for mi, (ms, msz) in enumerate(MT):
                        rsz = MT_REAL[mi]
                        e1 = act.tile([128, NCH], F32R, tag=f"e{mi}")
                        nc.scalar.activation(
                            e1[:rsz, :], p_er[:rsz, :], AF.Exp,
                            bias=whsb[mi][:rsz, 192 + j:193 + j].bitcast(F32),
                        )
                        e2 = act.tile([128, NCH], F32, tag="e2")
                        nc.scalar.activation(
                            e2[:rsz, :], p_er[:rsz, :], AF.Exp, scale=0.2,
                            bias=whsb[mi][:rsz, 196 + j:197 + j].bitcast(F32),
                        )
                        nc.vector.tensor_max(
                            e1[:rsz, :], e1[:rsz, :].bitcast(F32), e2[:rsz, :]
                        )
                        e_tiles.append(e1)
                    p_s = ps.tile([1, NCH], F32, tag="row")
                    for mi in range(3):
                        rsz = MT_REAL[mi]
                        nc.tensor.matmul(
                            p_s[:], ones_col[:rsz, :], e_tiles[mi][:rsz, :],
                            start=(mi == 0), stop=(mi == 2),
                        )
                    r_row = act.tile([1, NCH], F32R, tag="r_row")
                    with nc.allow_low_precision("f32r rounding is fine"):
                        nc.vector.reciprocal(r_row[:], p_s[:])
                    p_rb2 = ps.tile([DH, NCH], F32, tag="bc")
                    nc.tensor.matmul(
                        p_rb2[:], ones_row[:, :DH], r_row[:], start=True, stop=True
                    )
                    rb_sb = act.tile([DH, NCH], F32, tag="rb_sb")
                    nc.scalar.copy(rb_sb[:], p_rb2[:])
                    p_agg = ps.tile([DH, NCH], F32, tag="mm")
                    for mi, (ms, msz) in enumerate(MT):
                        rsz = MT_REAL[mi]
                        nc.tensor.matmul(
                            p_agg[:],
                            whsb[mi][:rsz, j * DH:(j + 1) * DH],
                            e_tiles[mi][:rsz, :],
                            start=(mi == 0), stop=(mi == 2),
                        )
                    at = act.tile([DH, NCH], F32R, tag=f"aggt{j}_{b % 2}")
                    nc.vector.tensor_mul(at[:], p_agg[:], rb_sb[:])
                    aggt.append(at)

                # proj partial: [D, NCH] = sum_j wproj[j].T-slices @ aggt[j]
                def proj_consume(mi, p, _b=b, _l=l):
                    part = act.tile([128, NCH], F32, tag="part")
                    nc.vector.tensor_scalar(
                        part[:], p[:], bproj8_sb[:, mi:mi + 1], None, ALU.add
                    )
                    nc.sync.dma_start(
                        out=ar_in[_l, 0, _b][mi * 128:(mi + 1) * 128, :],
                        in_=part[:, 0:T],
                    )

                for mi in range(KD):
                    p = ps.tile([128, NCH], F32, tag="mm")
                    for j in range(HPC):
                        nc.tensor.matmul(
                            p[:], wp[j][:, mi * 128:(mi + 1) * 128], aggt[j][:],
                            start=(j == 0), stop=(j == HPC - 1),
                        )
                    proj_consume(mi, p)

                nc.gpsimd.collective_compute(
                    "AllReduce", ALU.add,
                    replica_groups=[list(range(NC))],
                    ins=[ar_in[l, 0, b][:].opt()],
                    outs=[ar_out[l, 0, b][:].opt()],
                )

            # xt += AR result (per batch), then sublayer 1
            for b in range(B):
                cs = b * NCH
                for k in range(KD):
                    tmp = act.tile([128, T], F32, tag="artmp")
                    nc.sync.dma_start(
                        out=tmp[:], in_=ar_out[l, 0, b][k * 128:(k + 1) * 128, :]
                    )
                    nc.vector.tensor_add(
                        xt[k][:, cs:cs + T], xt[k][:, cs:cs + T].bitcast(F32), tmp[:]
                    )

            # ---------- sublayer 1: FFN ----------
            wfc_sb = []
            for k in range(KD):
                t = wgt.tile([128, VL], F32R, tag=f"wbig{k}")
                nc.sync.dma_start(
                    out=t[:, 0:FFL], in_=d_wfc[l, k * 128:(k + 1) * 128, :]
                )
                wfc_sb.append(t)
            wout_sb = []
            for k in range(KF):
                t = wgt.tile([128, D], F32R, tag=f"wo{k}")
                nc.sync.dma_start(
                    out=t[:], in_=d_wout[l, k * 128:(k + 1) * 128, :]
                )
                wout_sb.append(t)

            for b in range(B):
                cs = b * NCH
                h2 = layer_norm(l, xt, ln2g_sb, ln2b_sb, b)
                g_tiles = []
                for mi in range(KF):
                    p = ps.tile([128, NCH], F32, tag="mm")
                    for k in range(KD):
                        nc.tensor.matmul(
                            p[:], wfc_sb[k][:, mi * 128:(mi + 1) * 128], h2[k][:],
                            start=(k == 0), stop=(k == KD - 1),
                        )
                    g = act.tile([128, NCH], F32R, tag=f"g{mi}")
                    nc.scalar.activation(
                        g[:], p[:], AF.Gelu, bias=bfc_sb[:, mi:mi + 1]
                    )
                    g_tiles.append(g)
                for mi in range(KD):
                    p = ps.tile([128, NCH], F32, tag="mm")
                    for k in range(KF):
                        nc.tensor.matmul(
                            p[:], wout_sb[k][:, mi * 128:(mi + 1) * 128], g_tiles[k][:],
                            start=(k == 0), stop=(k == KF - 1),
                        )
                    part = act.tile([128, NCH], F32, tag="part")
                    nc.vector.tensor_scalar(
                        part[:], p[:], bout8_sb[:, mi:mi + 1], None, ALU.add
                    )
                    nc.sync.dma_start(
                        out=ar_in[l, 1, b][mi * 128:(mi + 1) * 128, :],
                        in_=part[:, 0:T],
                    )
                nc.gpsimd.collective_compute(
                    "AllReduce", ALU.add,
                    replica_groups=[list(range(NC))],
                    ins=[ar_in[l, 1, b][:].opt()],
                    outs=[ar_out[l, 1, b][:].opt()],
                )

            for b in range(B):
                cs = b * NCH
                for k in range(KD):
                    tmp = act.tile([128, T], F32, tag="artmp")
                    nc.sync.dma_start(
                        out=tmp[:], in_=ar_out[l, 1, b][k * 128:(k + 1) * 128, :]
                    )
                    nc.vector.tensor_add(
                        xt[k][:, cs:cs + T], xt[k][:, cs:cs + T].bitcast(F32), tmp[:]
                    )

        # ---------- final LN + vocab-sharded head ----------
        lnfg_sb = load_gb(d_lnfg, None, KD, "ln1g")
        lnfb_sb = load_gb(d_lnfb, None, KD, "ln1b")
        for b in range(B):
            xf = layer_norm(0, xt, lnfg_sb, lnfb_sb, b)
            for mi in range(VL // 128):
                p = ps.tile([128, NCH], F32, tag="mm")
                for k in range(KD):
                    wt = wgt.tile([128, VL], F32R, tag=f"wbig{k}")
                    if b == 0 and mi == 0:
                        nc.sync.dma_start(
                            out=wt[:], in_=d_whead[k * 128:(k + 1) * 128, :]
                        )
                    nc.tensor.matmul(
                        p[:], wt[:, mi * 128:(mi + 1) * 128], xf[k][:],
                        start=(k == 0), stop=(k == KD - 1),
                    )
                lg = act.tile([128, NCH], F32, tag="part")
                nc.scalar.copy(lg[:], p[:])
                nc.sync.dma_start(
                    out=d_logits[mi * 128:(mi + 1) * 128, b * T:(b + 1) * T],
                    in_=lg[:, 0:T],
                )

        for p in reversed(list(pools.values())):
            pass
    nc.compile()
    return nc


def _get_nc():
    if "nc" not in _CACHE:
        _CACHE["nc"] = _build_nc()
    return _CACHE["nc"]


# --------------------------------------------------------------------------
# numpy fallback (exact reference semantics, any edge structure)
# --------------------------------------------------------------------------

def _numpy_forward(inp):
    from scipy.special import erf

    def ln(x, g, b):
        m = x.mean(-1, keepdims=True)
        v = ((x - m) ** 2).mean(-1, keepdims=True)
        return (x - m) / np.sqrt(v + EPS) * g + b

    f32 = np.float32
    objs_e = np.asarray(inp["obj_emb_w"])[np.asarray(inp["objs"])]
    pe = np.asarray(inp["poss_emb_w"])[np.asarray(inp["poss"])]
    nfeat = np.concatenate([objs_e, pe[:, :NOBJ], pe[:, NOBJ:]], axis=-1)
    z = np.asarray(inp["tok_emb"])[np.asarray(inp["z_indices"])]
    x = (np.concatenate([nfeat, z], axis=1) + np.asarray(inp["pos_emb"])[:, :T])
    x = x.reshape(N, D).astype(f32)
    src = np.asarray(inp["src"]).astype(np.int64)
    dst = np.asarray(inp["dst"]).astype(np.int64)
    for l in range(L):
        h = ln(x, inp["ln1_g"][l], inp["ln1_b"][l])
        Wh = (h @ np.asarray(inp["W_attn"][l])).reshape(N, H, DH)
        el = np.einsum("nhd,hd->nh", Wh, np.asarray(inp["a_l"][l]))
        er = np.einsum("nhd,hd->nh", Wh, np.asarray(inp["a_r"][l]))
        e = el[src] + er[dst]
        e = np.where(e >= 0, e, 0.2 * e)
        m = np.full((N, H), -np.inf, f32)
        np.maximum.at(m, dst, e)
        ex = np.exp(e - m[dst])
        s = np.zeros((N, H), f32)
        np.add.at(s, dst, ex)
        alpha = ex / s[dst]
        agg = np.zeros((N, H, DH), f32)
        np.add.at(agg, dst, alpha[:, :, None] * Wh[src])
        x = x + agg.reshape(N, D) @ np.asarray(inp["W_proj"][l]) + np.asarray(inp["b_proj"][l])
        h2 = ln(x, inp["ln2_g"][l], inp["ln2_b"][l])
        ff = h2 @ np.asarray(inp["W_fc"][l]) + np.asarray(inp["b_fc"][l])
        ff = ff * 0.5 * (1.0 + erf(ff / np.sqrt(2.0)))
        x = x + ff @ np.asarray(inp["W_out"][l]) + np.asarray(inp["b_out"][l])
    x = ln(x, inp["lnf_g"], inp["lnf_b"])
    return (x @ np.asarray(inp["head_w"])).reshape(B, T, V).astype(f32)


# --------------------------------------------------------------------------
# public entry point
# --------------------------------------------------------------------------

def _edges_are_block_diag(inp):
    src, dst = _block_diag_edges_np()
    return (
        np.asarray(inp["src"]).shape == src.shape
        and np.array_equal(np.asarray(inp["src"], np.int64), src)
        and np.array_equal(np.asarray(inp["dst"], np.int64), dst)
    )


def _run_device(in_maps):
    from concourse import bass2jax

    nc = _get_nc()
    results = bass2jax.run_bass_via_pjrt(nc, in_maps, n_cores=NC)
    return results


def _assemble(results):
    parts = [results[c]["logits"] for c in range(NC)]  # each [VL, N]
    full = np.concatenate(parts, axis=0)               # [V, N]
    return np.ascontiguousarray(full.T).reshape(B, T, V)


def kernel(**inputs):
    if not _edges_are_block_diag(inputs):
        return _numpy_forward(inputs)
    in_maps = _host_inputs(inputs)
    results = _run_device(in_maps)
    return _assemble(results)


# revision 10
# speedup vs baseline: 1.7067x; 1.7067x over previous
"""Trainium2 Bass kernel for nn_GAT_42786464203341.

8-way tensor parallel (Megatron-style) over one trn2 chip:
  - The GAT edges are block-diagonal fully-connected per sample, so message
    passing is dense per-sample attention with scores leaky(el[i] + er[j]),
    softmaxed over source i.  exp/softmax needs no max-subtraction (scores
    are O(1)).
  - Activations are feature-major (x^T: [D, nodes]) so they feed the PE
    array directly (contraction dim on partitions).  LayerNorm reductions
    over D (partitions) use ones-vector matmuls; per-node stats broadcast
    back via rank-1 (K=1) matmuls.
  - Attention is head-parallel (2 heads/core); W_proj row-sharded ->
    partial [D, nodes] -> AllReduce.  FFN column/row sharded -> AllReduce.
    Head is vocab-sharded; host concatenates the 8 logits slices.
  - Matmuls in float32r (tf32-ish, ~1.5e-4/mm) at full PE rate; node axis
    padded 530 -> 532 = 2 x 266 (f32r even-free-dim constraint).
  - 12 AllReduces (3 layers x 2 sublayers x 2 batches), batch-split so a
    collective overlaps the other batch's compute.
"""

import time
from contextlib import ExitStack

import numpy as np

import concourse.bass as bass
import concourse.tile as tile
from concourse import bacc, mybir
from concourse.masks import make_identity

F32 = mybir.dt.float32
F32R = mybir.dt.float32r

B, T, NOBJ = 2, 265, 9
D, H, DH = 1536, 16, 96
V, PV, L, FF = 8192, 512, 3, 6144
N = B * T          # 530
NC = 8             # cores
HPC = H // NC      # heads per core
FFL = FF // NC     # 768
VL = V // NC       # 1024
NCH = T + 1        # 266 (col 265 of each chunk is zero padding)
NP = B * NCH       # 532
KD = D // 128      # 12
KF = FFL // 128    # 6
MT = [(0, 128), (128, 128), (256, 10)]   # node tiles per batch (start, size)
MT_REAL = [128, 128, 9]                  # non-pad rows per node tile
EPS = 1e-5

_CACHE = {}


# --------------------------------------------------------------------------
# host-side input prep
# --------------------------------------------------------------------------

def _block_diag_edges_np():
    base = np.arange(T)
    src = np.concatenate([g * T + np.repeat(base, T) for g in range(B)])
    dst = np.concatenate([g * T + np.tile(base, T) for g in range(B)])
    return src.astype(np.int64), dst.astype(np.int64)


def _host_inputs(inp):
    f32 = np.float32
    objs_e = np.asarray(inp["obj_emb_w"])[np.asarray(inp["objs"])]
    pe = np.asarray(inp["poss_emb_w"])[np.asarray(inp["poss"])]
    nfeat = np.concatenate([objs_e, pe[:, :NOBJ], pe[:, NOBJ:]], axis=-1)
    z = np.asarray(inp["tok_emb"])[np.asarray(inp["z_indices"])]
    x0 = np.concatenate([nfeat, z], axis=1) + np.asarray(inp["pos_emb"])[:, :T]
    x0 = x0.reshape(N, D).astype(f32)

    x0t = np.zeros((D, NP), f32)
    for b in range(B):
        x0t[:, b * NCH:b * NCH + T] = x0[b * T:(b + 1) * T].T

    W_attn = np.asarray(inp["W_attn"], f32)
    a_l = np.asarray(inp["a_l"], f32)
    a_r = np.asarray(inp["a_r"], f32)
    W_proj = np.asarray(inp["W_proj"], f32)
    W_fc = np.asarray(inp["W_fc"], f32)
    W_out = np.asarray(inp["W_out"], f32)
    head_w = np.asarray(inp["head_w"], f32)

    def cols(vec, k_tiles):  # [3, D'] -> [3, 128, k_tiles]
        v = np.asarray(vec, f32)
        return np.transpose(v.reshape(3, k_tiles, 128), (0, 2, 1)).copy()

    ln1g, ln1b = cols(inp["ln1_g"], KD), cols(inp["ln1_b"], KD)
    ln2g, ln2b = cols(inp["ln2_g"], KD), cols(inp["ln2_b"], KD)
    lnfg = np.asarray(inp["lnf_g"], f32).reshape(KD, 128).T.copy()
    lnfb = np.asarray(inp["lnf_b"], f32).reshape(KD, 128).T.copy()
    bfc_all = cols(inp["b_fc"], KF * NC)
    bout8 = cols(np.asarray(inp["b_out"], f32) / NC, KD)
    bproj8 = cols(np.asarray(inp["b_proj"], f32) / NC, KD)

    maps = []
    for c in range(NC):
        h0 = c * HPC
        wattn = np.zeros((L, D, 256), f32)
        for j in range(HPC):
            hg = h0 + j
            blk = W_attn[:, :, hg * DH:(hg + 1) * DH]         # [3, D, DH]
            wattn[:, :, j * DH:(j + 1) * DH] = blk
            # el/er are linear in h: fold (W_attn-block @ a) into one column
            wattn[:, :, 192 + j] = np.matmul(blk, a_l[:, hg, :, None])[..., 0]
            wattn[:, :, 194 + j] = np.matmul(blk, a_r[:, hg, :, None])[..., 0]
        wproj = np.stack(
            [W_proj[:, (h0 + j) * DH:(h0 + j + 1) * DH, :] for j in range(HPC)],
            axis=1,
        )
        maps.append({
            "x0t": x0t,
            "wattn": wattn,
            "wproj": np.ascontiguousarray(wproj),
            "wfc": np.ascontiguousarray(W_fc[:, :, c * FFL:(c + 1) * FFL]),
            "wout": np.ascontiguousarray(W_out[:, c * FFL:(c + 1) * FFL, :]),
            "whead": np.ascontiguousarray(head_w[:, c * VL:(c + 1) * VL]),
            "ones_col": np.ones((128, 1), f32),
            "ones_row": np.ones((1, 128), f32),
            "ln1g": ln1g, "ln1b": ln1b, "ln2g": ln2g, "ln2b": ln2b,
            "lnfg": lnfg, "lnfb": lnfb,
            "bfc": np.ascontiguousarray(bfc_all[:, :, c * KF:(c + 1) * KF]),
            "bout8": bout8, "bproj8": bproj8,
        })
    return maps


# --------------------------------------------------------------------------
# device program
# --------------------------------------------------------------------------

def _build_nc():
    nc = bacc.Bacc("TRN2", target_bir_lowering=False, debug=False, num_devices=NC)

    d_x0t = nc.declare_dram_parameter("x0t", [D, NP], F32R, isOutput=False)
    d_wattn = nc.declare_dram_parameter("wattn", [L, D, 256], F32R, isOutput=False)
    d_wproj = nc.declare_dram_parameter("wproj", [L, HPC, DH, D], F32R, isOutput=False)
    d_wfc = nc.declare_dram_parameter("wfc", [L, D, FFL], F32R, isOutput=False)
    d_wout = nc.declare_dram_parameter("wout", [L, FFL, D], F32R, isOutput=False)
    d_whead = nc.declare_dram_parameter("whead", [D, VL], F32R, isOutput=False)
    d_ones_col = nc.declare_dram_parameter("ones_col", [128, 1], F32R, isOutput=False)
    d_ones_row = nc.declare_dram_parameter("ones_row", [1, 128], F32R, isOutput=False)
    d_ln1g = nc.declare_dram_parameter("ln1g", [L, 128, KD], F32, isOutput=False)
    d_ln1b = nc.declare_dram_parameter("ln1b", [L, 128, KD], F32, isOutput=False)
    d_ln2g = nc.declare_dram_parameter("ln2g", [L, 128, KD], F32, isOutput=False)
    d_ln2b = nc.declare_dram_parameter("ln2b", [L, 128, KD], F32, isOutput=False)
    d_lnfg = nc.declare_dram_parameter("lnfg", [128, KD], F32, isOutput=False)
    d_lnfb = nc.declare_dram_parameter("lnfb", [128, KD], F32, isOutput=False)
    d_bfc = nc.declare_dram_parameter("bfc", [L, 128, KF], F32, isOutput=False)
    d_bout8 = nc.declare_dram_parameter("bout8", [L, 128, KD], F32, isOutput=False)
    d_bproj8 = nc.declare_dram_parameter("bproj8", [L, 128, KD], F32, isOutput=False)
    d_logits = nc.declare_dram_parameter("logits", [VL, N], F32, isOutput=True)

    ar_in, ar_out = {}, {}
    for l in range(L):
        for s in range(2):
            for b in range(B):
                ar_in[l, s, b] = nc.dram_tensor(f"arin_{l}_{s}_{b}", [D, T], F32)
                ar_out[l, s, b] = nc.dram_tensor(
                    f"arout_{l}_{s}_{b}", [D, T], F32, addr_space="Shared"
                )

    AF = mybir.ActivationFunctionType
    ALU = mybir.AluOpType

    with tile.TileContext(nc) as tc, ExitStack() as ctx:
        res = ctx.enter_context(tc.tile_pool(name="res", bufs=1))
        cst = ctx.enter_context(tc.tile_pool(name="cst", bufs=2))
        a1 = ctx.enter_context(tc.tile_pool(name="a1", bufs=1))
        a2 = ctx.enter_context(tc.tile_pool(name="a2", bufs=2))
        a3 = ctx.enter_context(tc.tile_pool(name="a3", bufs=3))
        wgt = ctx.enter_context(tc.tile_pool(name="wgt", bufs=1))
        ps2 = ctx.enter_context(tc.tile_pool(name="ps2", bufs=2, space="PSUM"))
        ps3 = ctx.enter_context(tc.tile_pool(name="ps3", bufs=3, space="PSUM"))

        # ---- persistent tiles
        xt = []
        for k in range(KD):
            t = res.tile([128, NP], F32R, tag=f"xt{k}")
            nc.sync.dma_start(out=t[:], in_=d_x0t[k * 128:(k + 1) * 128, :])
            xt.append(t)
        ones_col = res.tile([128, 1], F32R, tag="ones_col")
        nc.sync.dma_start(out=ones_col[:], in_=d_ones_col[:])
        ones_row = res.tile([1, 128], F32R, tag="ones_row")
        nc.sync.dma_start(out=ones_row[:], in_=d_ones_row[:])
        ident = res.tile([128, 128], F32, tag="ident")
        make_identity(nc, ident[:])
        eps_col = res.tile([1, 1], F32, tag="eps")
        nc.vector.memset(eps_col[:], EPS)

        def load_gb(dram, l, ktiles, tag):
            t = cst.tile([128, ktiles], F32, tag=tag)
            nc.sync.dma_start(out=t[:], in_=dram[l] if l is not None else dram[:])
            return t

        def layer_norm(g_sb, b_sb, b):
            """feature-major LN of xt batch-chunk b -> 12 f32r tiles [128, 266]"""
            cs = b * NCH
            p_sums = ps2.tile([1, NCH], F32, tag="row")
            for k in range(KD):
                nc.tensor.matmul(
                    p_sums[:], ones_col[:], xt[k][:, cs:cs + NCH],
                    start=(k == 0), stop=(k == KD - 1),
                )
            p_sqs = ps2.tile([1, NCH], F32, tag="row")
            for k in range(KD):
                sq = a2.tile([128, NCH], F32R, tag="sq")
                nc.scalar.activation(
                    sq[:], xt[k][:, cs:cs + NCH].bitcast(F32), AF.Square
                )
                nc.tensor.matmul(
                    p_sqs[:], ones_col[:], sq[:],
                    start=(k == 0), stop=(k == KD - 1),
                )
            m_row = a1.tile([1, NCH], F32R, tag="m_row")
            nc.vector.tensor_scalar(m_row[:], p_sums[:], 1.0 / D, None, ALU.mult)
            ms = a1.tile([1, NCH], F32, tag="ms_row")
            nc.scalar.activation(ms[:], m_row[:].bitcast(F32), AF.Square)
            var = a1.tile([1, NCH], F32, tag="var_row")
            nc.vector.tensor_scalar(var[:], p_sqs[:], 1.0 / D, None, ALU.mult)
            nc.vector.tensor_sub(var[:], var[:], ms[:])
            std = a1.tile([1, NCH], F32, tag="std_row")
            nc.scalar.activation(std[:], var[:], AF.Sqrt, bias=eps_col[:])
            rs_row = a1.tile([1, NCH], F32R, tag="rs_row")
            with nc.allow_low_precision("f32r rounding"):
                nc.vector.reciprocal(rs_row[:], std[:])
            mr_row = a1.tile([1, NCH], F32R, tag="mr_row")
            with nc.allow_low_precision("f32r rounding"):
                nc.vector.tensor_mul(
                    mr_row[:], m_row[:].bitcast(F32), rs_row[:].bitcast(F32)
                )
            p_rb = ps3.tile([128, NCH], F32, tag="bc")
            nc.tensor.matmul(p_rb[:], ones_row[:], rs_row[:], start=True, stop=True)
            p_mb = ps3.tile([128, NCH], F32, tag="bc")
            nc.tensor.matmul(p_mb[:], ones_row[:], mr_row[:], start=True, stop=True)
            h_tiles = []
            for k in range(KD):
                t1 = a2.tile([128, NCH], F32, tag="lnt")
                nc.vector.tensor_mul(
                    t1[:], xt[k][:, cs:cs + NCH].bitcast(F32), p_rb[:]
                )
                nc.vector.tensor_sub(t1[:], t1[:], p_mb[:])
                h = a1.tile([128, NCH], F32R, tag=f"h{k}")
                nc.scalar.activation(
                    h[:], t1[:], AF.Identity,
                    bias=b_sb[:, k:k + 1], scale=g_sb[:, k:k + 1],
                )
                h_tiles.append(h)
            return h_tiles

        # ================= network =================
        for l in range(L):
            ln1g_sb = load_gb(d_ln1g, l, KD, "ln1g")
            ln1b_sb = load_gb(d_ln1b, l, KD, "ln1b")
            ln2g_sb = load_gb(d_ln2g, l, KD, "ln2g")
            ln2b_sb = load_gb(d_ln2b, l, KD, "ln2b")
            bfc_sb = load_gb(d_bfc, l, KF, "bfc")
            bout8_sb = load_gb(d_bout8, l, KD, "bout8")
            bproj8_sb = load_gb(d_bproj8, l, KD, "bproj8")

            wa = []
            for k in range(KD):
                t = wgt.tile([128, 256], F32R, tag=f"wa{k}")
                nc.sync.dma_start(out=t[:], in_=d_wattn[l, k * 128:(k + 1) * 128, :])
                wa.append(t)
            wp = []
            for j in range(HPC):
                t = wgt.tile([DH, D], F32R, tag=f"wp{j}")
                nc.sync.dma_start(out=t[:], in_=d_wproj[l, j])
                wp.append(t)

            # ---------- attention sublayer ----------
            for b in range(B):
                h_tiles = layer_norm(ln1g_sb, ln1b_sb, b)

                whsb = []
                for mi, (ms, msz) in enumerate(MT):
                    p = ps3.tile([128, 256], F32, tag="mm")
                    for k in range(KD):
                        nc.tensor.matmul(
                            p[:msz, :], h_tiles[k][:, ms:ms + msz], wa[k][:],
                            start=(k == 0), stop=(k == KD - 1),
                        )
                    w = a1.tile([128, 198], F32R, tag=f"whsb{mi}_{b}")
                    nc.scalar.copy(w[:msz, 0:196], p[:msz, 0:196])
                    nc.vector.tensor_scalar(
                        w[:msz, 196:198], p[:msz, 192:194], 0.2, None, ALU.mult
                    )
                    whsb.append(w)

                erow = [
                    a1.tile([1, NCH], F32R, name=f"erow{j}_{b}", tag=f"erow{j}_{b}")
                    for j in range(HPC)
                ]
                for mi, (ms, msz) in enumerate(MT):
                    for j in range(HPC):
                        pt = ps2.tile([1, 128], F32, tag="row")
                        nc.tensor.transpose(
                            pt[:, :msz],
                            whsb[mi][:msz, 194 + j:195 + j].bitcast(F32),
                            ident[:msz, :msz],
                        )
                        nc.scalar.copy(erow[j][:, ms:ms + msz], pt[:, :msz])

                aggt = []
                for j in range(HPC):
                    p_er = ps3.tile([128, NCH], F32, tag="bc")
                    nc.tensor.matmul(
                        p_er[:], ones_row[:], erow[j][:], start=True, stop=True
                    )
                    e_tiles = []
                    for mi in range(3):
                        rsz = MT_REAL[mi]
                        e1 = a2.tile([128, NCH], F32R, tag=f"e{mi}")
                        nc.scalar.activation(
                            e1[:rsz, :], p_er[:rsz, :], AF.Exp,
                            bias=whsb[mi][:rsz, 192 + j:193 + j].bitcast(F32),
                        )
                        e2 = a1.tile([128, NCH], F32, tag="e2")
                        nc.scalar.activation(
                            e2[:rsz, :], p_er[:rsz, :], AF.Exp, scale=0.2,
                            bias=whsb[mi][:rsz, 196 + j:197 + j].bitcast(F32),
                        )
                        nc.vector.tensor_max(
                            e1[:rsz, :], e1[:rsz, :].bitcast(F32), e2[:rsz, :]
                        )
                        e_tiles.append(e1)
                    p_s = ps2.tile([1, NCH], F32, tag="row")
                    for mi in range(3):
                        rsz = MT_REAL[mi]
                        nc.tensor.matmul(
                            p_s[:], ones_col[:rsz, :], e_tiles[mi][:rsz, :],
                            start=(mi == 0), stop=(mi == 2),
                        )
                    r_row = a1.tile([1, NCH], F32R, tag="r_row")
                    with nc.allow_low_precision("f32r rounding"):
                        nc.vector.reciprocal(r_row[:], p_s[:])
                    p_rb2 = ps3.tile([DH, NCH], F32, tag="bc")
                    nc.tensor.matmul(
                        p_rb2[:], ones_row[:, :DH], r_row[:], start=True, stop=True
                    )
                    rb_sb = a1.tile([DH, NCH], F32, tag="rb_sb")
                    nc.scalar.copy(rb_sb[:], p_rb2[:])
                    p_agg = ps3.tile([DH, NCH], F32, tag="mm")
                    for mi in range(3):
                        rsz = MT_REAL[mi]
                        nc.tensor.matmul(
                            p_agg[:],
                            whsb[mi][:rsz, j * DH:(j + 1) * DH],
                            e_tiles[mi][:rsz, :],
                            start=(mi == 0), stop=(mi == 2),
                        )
                    at = a1.tile([DH, NCH], F32R, tag=f"aggt{j}_{b}")
                    nc.vector.tensor_mul(at[:], p_agg[:], rb_sb[:])
                    aggt.append(at)

                for mi in range(KD):
                    p = ps3.tile([128, NCH], F32, tag="mm")
                    for j in range(HPC):
                        nc.tensor.matmul(
                            p[:], wp[j][:, mi * 128:(mi + 1) * 128], aggt[j][:],
                            start=(j == 0), stop=(j == HPC - 1),
                        )
                    part = a3.tile([128, NCH], F32, tag="part")
                    nc.vector.tensor_scalar(
                        part[:], p[:], bproj8_sb[:, mi:mi + 1], None, ALU.add
                    )
                    nc.sync.dma_start(
                        out=ar_in[l, 0, b][mi * 128:(mi + 1) * 128, :],
                        in_=part[:, 0:T],
                    )
                nc.gpsimd.collective_compute(
                    "AllReduce", ALU.add,
                    replica_groups=[list(range(NC))],
                    ins=[ar_in[l, 0, b][:].opt()],
                    outs=[ar_out[l, 0, b][:].opt()],
                )

            for b in range(B):
                cs = b * NCH
                for k in range(KD):
                    tmp = a3.tile([128, T], F32, tag="artmp")
                    nc.sync.dma_start(
                        out=tmp[:], in_=ar_out[l, 0, b][k * 128:(k + 1) * 128, :]
                    )
                    nc.vector.tensor_add(
                        xt[k][:, cs:cs + T], xt[k][:, cs:cs + T].bitcast(F32), tmp[:]
                    )

            # ---------- FFN sublayer ----------
            wfc_sb = []
            for k in range(KD):
                t = wgt.tile([128, VL], F32R, tag=f"wbig{k}")
                nc.sync.dma_start(
                    out=t[:, 0:FFL], in_=d_wfc[l, k * 128:(k + 1) * 128, :]
                )
                wfc_sb.append(t)
            wout_sb = []
            for k in range(KF):
                t = wgt.tile([128, D], F32R, tag=f"wo{k}")
                nc.sync.dma_start(out=t[:], in_=d_wout[l, k * 128:(k + 1) * 128, :])
                wout_sb.append(t)

            for b in range(B):
                h2 = layer_norm(ln2g_sb, ln2b_sb, b)
                g_tiles = []
                for mi in range(KF):
                    p = ps3.tile([128, NCH], F32, tag="mm")
                    for k in range(KD):
                        nc.tensor.matmul(
                            p[:], wfc_sb[k][:, mi * 128:(mi + 1) * 128], h2[k][:],
                            start=(k == 0), stop=(k == KD - 1),
                        )
                    g = a2.tile([128, NCH], F32R, tag=f"g{mi}")
                    nc.scalar.activation(g[:], p[:], AF.Gelu, bias=bfc_sb[:, mi:mi + 1])
                    g_tiles.append(g)
                for mi in range(KD):
                    p = ps3.tile([128, NCH], F32, tag="mm")
                    for k in range(KF):
                        nc.tensor.matmul(
                            p[:], wout_sb[k][:, mi * 128:(mi + 1) * 128], g_tiles[k][:],
                            start=(k == 0), stop=(k == KF - 1),
                        )
                    part = a3.tile([128, NCH], F32, tag="part")
                    nc.vector.tensor_scalar(
                        part[:], p[:], bout8_sb[:, mi:mi + 1], None, ALU.add
                    )
                    nc.sync.dma_start(
                        out=ar_in[l, 1, b][mi * 128:(mi + 1) * 128, :],
                        in_=part[:, 0:T],
                    )
                nc.gpsimd.collective_compute(
                    "AllReduce", ALU.add,
                    replica_groups=[list(range(NC))],
                    ins=[ar_in[l, 1, b][:].opt()],
                    outs=[ar_out[l, 1, b][:].opt()],
                )

            for b in range(B):
                cs = b * NCH
                for k in range(KD):
                    tmp = a3.tile([128, T], F32, tag="artmp")
                    nc.sync.dma_start(
                        out=tmp[:], in_=ar_out[l, 1, b][k * 128:(k + 1) * 128, :]
                    )
                    nc.vector.tensor_add(
                        xt[k][:, cs:cs + T], xt[k][:, cs:cs + T].bitcast(F32), tmp[:]
                    )

        # ---------- final LN + vocab-sharded head ----------
        lnfg_sb = load_gb(d_lnfg, None, KD, "ln1g")
        lnfb_sb = load_gb(d_lnfb, None, KD, "ln1b")
        wh_sb = []
        for k in range(KD):
            t = wgt.tile([128, VL], F32R, tag=f"wbig{k}")
            nc.sync.dma_start(out=t[:], in_=d_whead[k * 128:(k + 1) * 128, :])
            wh_sb.append(t)
        for b in range(B):
            xf = layer_norm(lnfg_sb, lnfb_sb, b)
            for mi in range(VL // 128):
                p = ps3.tile([128, NCH], F32, tag="mm")
                for k in range(KD):
                    nc.tensor.matmul(
                        p[:], wh_sb[k][:, mi * 128:(mi + 1) * 128], xf[k][:],
                        start=(k == 0), stop=(k == KD - 1),
                    )
                lg = a3.tile([128, NCH], F32, tag="part")
                nc.scalar.copy(lg[:], p[:])
                nc.sync.dma_start(
                    out=d_logits[mi * 128:(mi + 1) * 128, b * T:(b + 1) * T],
                    in_=lg[:, 0:T],
                )

    nc.compile()
    return nc


def _get_nc():
    if "nc" not in _CACHE:
        _CACHE["nc"] = _build_nc()
    return _CACHE["nc"]


# --------------------------------------------------------------------------
# numpy fallback (exact reference semantics for arbitrary edges)
# --------------------------------------------------------------------------

def _numpy_forward(inp):
    from scipy.special import erf

    def ln(x, g, b):
        m = x.mean(-1, keepdims=True)
        v = ((x - m) ** 2).mean(-1, keepdims=True)
        return (x - m) / np.sqrt(v + EPS) * g + b

    f32 = np.float32
    objs_e = np.asarray(inp["obj_emb_w"])[np.asarray(inp["objs"])]
    pe = np.asarray(inp["poss_emb_w"])[np.asarray(inp["poss"])]
    nfeat = np.concatenate([objs_e, pe[:, :NOBJ], pe[:, NOBJ:]], axis=-1)
    z = np.asarray(inp["tok_emb"])[np.asarray(inp["z_indices"])]
    x = np.concatenate([nfeat, z], axis=1) + np.asarray(inp["pos_emb"])[:, :T]
    x = x.reshape(N, D).astype(f32)
    src = np.asarray(inp["src"]).astype(np.int64)
    dst = np.asarray(inp["dst"]).astype(np.int64)
    for l in range(L):
        h = ln(x, inp["ln1_g"][l], inp["ln1_b"][l])
        Wh = (h @ np.asarray(inp["W_attn"][l])).reshape(N, H, DH)
        el = np.einsum("nhd,hd->nh", Wh, np.asarray(inp["a_l"][l]))
        er = np.einsum("nhd,hd->nh", Wh, np.asarray(inp["a_r"][l]))
        e = el[src] + er[dst]
        e = np.where(e >= 0, e, 0.2 * e)
        m = np.full((N, H), -np.inf, f32)
        np.maximum.at(m, dst, e)
        m[~np.isfinite(m)] = 0.0
        ex = np.exp(e - m[dst])
        s = np.zeros((N, H), f32)
        np.add.at(s, dst, ex)
        alpha = ex / s[dst]
        agg = np.zeros((N, H, DH), f32)
        np.add.at(agg, dst, alpha[:, :, None] * Wh[src])
        x = x + agg.reshape(N, D) @ np.asarray(inp["W_proj"][l]) \
            + np.asarray(inp["b_proj"][l])
        h2 = ln(x, inp["ln2_g"][l], inp["ln2_b"][l])
        ff = h2 @ np.asarray(inp["W_fc"][l]) + np.asarray(inp["b_fc"][l])
        ff = ff * 0.5 * (1.0 + erf(ff / np.sqrt(2.0)))
        x = x + ff @ np.asarray(inp["W_out"][l]) + np.asarray(inp["b_out"][l])
    x = ln(x, inp["lnf_g"], inp["lnf_b"])
    return (x @ np.asarray(inp["head_w"])).reshape(B, T, V).astype(f32)


# --------------------------------------------------------------------------
# public entry
# --------------------------------------------------------------------------

def _edges_are_block_diag(inp):
    src, dst = _block_diag_edges_np()
    s = np.asarray(inp["src"])
    d = np.asarray(inp["dst"])
    return (
        s.shape == src.shape
        and np.array_equal(s.astype(np.int64), src)
        and np.array_equal(d.astype(np.int64), dst)
    )


def _assemble(results):
    full = np.concatenate([results[c]["logits"] for c in range(NC)], axis=0)
    return np.ascontiguousarray(full.T).reshape(B, T, V)


def kernel(**inputs):
    if not _edges_are_block_diag(inputs):
        return _numpy_forward(inputs)
    from concourse import bass2jax

    in_maps = _host_inputs(inputs)
    results = bass2jax.run_bass_via_pjrt(_get_nc(), in_maps, n_cores=NC)
    return _assemble(results)


# --------------------------------------------------------------------------
# benchmarking (repeated execution, device-resident inputs)
# --------------------------------------------------------------------------

def _make_runner(nc):
    """Persistent jitted shard_map callable for nc (multi-core), mirroring
    bass2jax.run_bass_via_pjrt but reusable across calls."""
    import jax
    from jax.sharding import Mesh, PartitionSpec
    from jax.experimental.shard_map import shard_map
    from concourse import bass2jax, mybir as _mybir

    bass2jax.install_neuronx_cc_hook()
    partition_name = nc.partition_id_tensor.name if nc.partition_id_tensor else None
    in_names, out_names, out_avals, zero_outs = [], [], [], []
    for alloc in nc.m.functions[0].allocations:
        if not isinstance(alloc, _mybir.MemoryLocationSet):
            continue
        name = alloc.memorylocations[0].name
        if alloc.kind == "ExternalInput":
            if name != partition_name:
                in_names.append(name)
        elif alloc.kind == "ExternalOutput":
            shape = tuple(alloc.tensor_shape)
            dtype = _mybir.dt.np(alloc.dtype)
            out_names.append(name)
            out_avals.append(jax.core.ShapedArray(shape, dtype))
            zero_outs.append(np.zeros(shape, dtype))
    n_params = len(in_names)
    all_in_names = list(in_names) + list(out_names)
    if partition_name is not None:
        all_in_names.append(partition_name)

    def _body(*args):
        operands = list(args)
        if partition_name is not None:
            operands.append(bass2jax.partition_id_tensor())
        return tuple(
            bass2jax._bass_exec_p.bind(
                *operands,
                out_avals=tuple(out_avals),
                in_names=tuple(all_in_names),
                out_names=tuple(out_names),
                lowering_input_output_aliases=(),
                sim_require_finite=True,
                sim_require_nnan=True,
                nc=nc,
            )
        )

    devices = jax.devices()[:NC]
    mesh = Mesh(np.asarray(devices), ("core",))
    n_outs = len(out_names)
    in_specs = (PartitionSpec("core"),) * (n_params + n_outs)
    out_specs = (PartitionSpec("core"),) * n_outs
    donate = tuple(range(n_params, n_params + n_outs))
    fn = jax.jit(
        shard_map(_body, mesh=mesh, in_specs=in_specs, out_specs=out_specs,
                  check_rep=False),
        donate_argnums=donate, keep_unused=True,
    )
    return fn, in_names, out_names, zero_outs, mesh


def _noop_nc():
    if "noop" in _CACHE:
        return _CACHE["noop"]
    nc = bacc.Bacc("TRN2", target_bir_lowering=False, debug=False, num_devices=NC)
    d_i = nc.declare_dram_parameter("ni", [128, 16], F32, isOutput=False)
    d_o = nc.declare_dram_parameter("no", [128, 16], F32, isOutput=True)
    with tile.TileContext(nc) as tc, ExitStack() as ctx:
        p = ctx.enter_context(tc.tile_pool(name="p", bufs=1))
        t = p.tile([128, 16], F32, tag="t")
        nc.sync.dma_start(out=t[:], in_=d_i[:])
        nc.sync.dma_start(out=d_o[:], in_=t[:])
    nc.compile()
    _CACHE["noop"] = nc
    return nc


def _timed_run(nc, in_maps, iters):
    """Median wall time (s) per execution with device-resident inputs."""
    import jax

    fn, in_names, out_names, zero_outs, mesh = _make_runner(nc)
    concat_in = [
        np.concatenate([np.asarray(m[name]) for m in in_maps], axis=0)
        for name in in_names
    ]
    dev_in = [jax.device_put(a) for a in concat_in]

    def zeros():
        return [
            np.zeros((NC * z.shape[0], *z.shape[1:]), z.dtype) for z in zero_outs
        ]

    outs = fn(*dev_in, *zeros())  # warm-up/compile
    jax.block_until_ready(outs)
    times = []
    for _ in range(iters):
        zs = zeros()
        t0 = time.perf_counter()
        outs = fn(*dev_in, *zs)
        jax.block_until_ready(outs)
        times.append(time.perf_counter() - t0)
    return float(np.median(times)), outs, out_names


def bench(inputs, iters=6):
    """Returns estimated HW kernel ns (dispatch-corrected wall time)."""
    in_maps = _host_inputs(inputs)
    t_full, _, _ = _timed_run(_get_nc(), in_maps, iters)
    noop_maps = [{"ni": np.zeros((128, 16), np.float32)} for _ in range(NC)]
    t_noop, _, _ = _timed_run(_noop_nc(), noop_maps, iters)
    print(f"  wall/iter full: {t_full * 1e6:.0f} us,  noop: {t_noop * 1e6:.0f} us")
    return max(t_full - t_noop, 0.0) * 1e9
